# revision 30
# baseline (speedup 1.0000x reference)
"""Trainium2 Bass kernel for batched GNN message passing.

Computes, for each batch b:
    ax[b] = segment_sum(x[b][cols] * vals[:, None], rows, num_segments=N_OUT)
    out[b] = ax[b] @ weight + bias

Strategy (8 NeuronCores):
  * The two batches share one graph, so x is interleaved host-side into
    xi[n] = concat(x[0,n], x[1,n]) -> [N_IN, 2*IN_C] fp16; one gathered
    row serves both batches.
  * Output rows are split into 128-row blocks, dealt to the 8 cores so
    every core runs an identical program: NB block-slots, slot s
    processing TPBL[s] + TPBH[s] tiles of 128 edges (dma_gather indices
    are int16, so edges are split into col < 32768 gathered from xi[0:]
    and col >= 32768 gathered from xi[32768:]).
  * Per 128-edge tile: dma_gather of 128 rows (1KB fp16 each) from HBM,
    then per 128-channel chunk a PE matmul pacc[ch, r] += G^T @ S^T
    accumulated over the slot's tiles (the segment sum, kept CHANNEL-
    MAJOR).  The scaled selection matrices S^T[e, r] = vals[e] *
    (rloc[e] == r) are PRECOMPUTED ON HOST in fp16 and streamed from HBM
    (dense DMA), because per-tile DVE builds cost ~5us each on HW and
    serialized the whole kernel.
  * Per slot epilogue: one PSUM->SBUF copy; the channel-major layout
    feeds the weight projection directly as lhsT (no transposes), then
    bias add (DVE) and DMA out.
"""

import math
import os as _os

import numpy as np

# ---------------------------------------------------------------- problem dims
B = 2
N_IN = 50000
N_OUT = 12500
NNZ = 500000
IN_C = 256
OUT_C = 256
N_CORES = 8
PB = 128  # rows per output block == partition count
H16 = 32768  # int16 index limit for dma_gather

GCH = int(_os.environ.get("K_GCH", "8"))  # 128-edge tiles per dma_gather call
# SWDGE queues: gather descriptor-gen spread over NSWQ Q7 core pairs
NSWQ = int(_os.environ.get("K_NSWQ", "4"))

_CACHE = {}
LAST_RESULTS = None


# ---------------------------------------------------------------- host planning
def _plan(rows, cols):
    """Pack output rows into (core, slot) bins of <=128 rows, balancing the
    low/high edge loads so every slot's tile counts are tight and uniform.
    Slot s everywhere holds TPBL[s] low tiles + TPBH[s] high tiles."""
    Lr = np.bincount(rows[cols < H16], minlength=N_OUT)
    Hr = np.bincount(rows[cols >= H16], minlength=N_OUT)
    NB = -(-N_OUT // (PB * N_CORES))
    nbins = N_CORES * NB
    tL = max(Lr.sum() / nbins, 1.0)
    tH = max(Hr.sum() / nbins, 1.0)

    order = np.argsort(-(Lr + Hr), kind="stable")
    binL = np.zeros(nbins)
    binH = np.zeros(nbins)
    binN = np.zeros(nbins, dtype=np.int64)
    bin_rows = [[] for _ in range(nbins)]
    for r in order:
        score = np.maximum((binL + Lr[r]) / tL, (binH + Hr[r]) / tH)
        score[binN >= PB] = np.inf
        b = int(score.argmin())
        binL[b] += Lr[r]
        binH[b] += Hr[r]
        binN[b] += 1
        bin_rows[b].append(int(r))

    # group bins into slots by (L-quantum, H) so per-slot maxima stay tight
    q = np.lexsort((-binH, -(-(-binL.astype(np.int64) // PB))))
    rowsets = [[None] * NB for _ in range(N_CORES)]
    for s in range(NB):
        grp = q[s * N_CORES : (s + 1) * N_CORES]
        for c, b in enumerate(grp):
            rowsets[c][s] = np.array(sorted(bin_rows[b]), dtype=np.int64)

    # tile counts from DISTINCT cols per (core, slot): repeated cols within
    # a slot are folded into one gathered row (S accumulates their vals)
    order_r = np.argsort(rows, kind="stable")
    bnd_r = np.searchsorted(rows[order_r], np.arange(N_OUT + 1))
    TPBL, TPBH = [], []
    for s in range(NB):
        tl = th = 0
        for c in range(N_CORES):
            lo, _, hi, _ = _slot_edges(rowsets[c][s], rows, cols, order_r,
                                       bnd_r)
            nlo = len(np.unique(cols[lo])) if len(lo) else 0
            nhi = len(np.unique(cols[hi])) if len(hi) else 0
            tl = max(tl, -(-nlo // PB))
            th = max(th, -(-nhi // PB))
        if tl + th == 0:
            tl = 1
        TPBL.append(tl)
        TPBH.append(th)
    return NB, TPBL, TPBH, rowsets


def _slot_edges(rowlist, rows, cols, order_r, bnd_r):
    """Edge ids + local row indices for one (core, slot), split lo/hi."""
    if rowlist is None or len(rowlist) == 0:
        e = np.empty(0, np.int64)
        return e, e, e, e
    per_row = [order_r[bnd_r[r] : bnd_r[r + 1]] for r in rowlist]
    eids = (np.concatenate(per_row) if per_row else np.empty(0, np.int64))
    rloc = np.repeat(np.arange(len(rowlist)), [len(e) for e in per_row])
    m = cols[eids] < H16
    return eids[m], rloc[m], eids[~m], rloc[~m]


def _wrap16(flat):
    """int16 index stream -> dma_gather layout: idx i at partition i%16,
    col i//16, replicated across the 8 gpsimd core groups."""
    n = len(flat)
    assert n % 16 == 0
    w = np.ascontiguousarray(flat.reshape(n // 16, 16).T.astype(np.int16))
    return np.ascontiguousarray(np.tile(w, (8, 1)))


def _pack_core(c, plan, rows, cols, vals, bias, order_r, bnd_r):
    """Per-core arrays: sT [128, NT*128] fp16 selection matrices (tile j at
    columns j*128:(j+1)*128, partition = edge-in-tile), wrapped int16
    gather index streams, bias."""
    NB, TPBL, TPBH, rowsets = plan
    NT = sum(TPBL) + sum(TPBH)
    s_acc = np.zeros(NT * PB * PB, dtype=np.float32)
    lowE, highE = [], []
    bias_c = np.zeros((NB * PB, OUT_C), dtype=np.float32)
    pos = 0
    for s in range(NB):
        rowlist = rowsets[c][s]
        nr = len(rowlist)
        lo, lo_rl, hi, hi_rl = _slot_edges(rowlist, rows, cols, order_r,
                                           bnd_r)
        if nr:
            bias_c[s * PB : s * PB + nr] = bias[rowlist]
        for lst, rl, tpb, base, acc in (
            (lo, lo_rl, TPBL[s], 0, lowE),
            (hi, hi_rl, TPBH[s], H16, highE),
        ):
            k = tpb * PB
            if k == 0:
                assert len(lst) == 0
                continue
            # gather DISTINCT cols once; S row accumulates all edge vals
            dcols, inv = np.unique(cols[lst], return_inverse=True)
            ne = len(dcols)
            assert ne <= k, (ne, k)
            np.add.at(s_acc, (pos + inv) * PB + rl, vals[lst])
            cc = np.zeros(k, dtype=np.int64)
            cc[:ne] = dcols - base
            acc.append(cc)
            pos += k
    assert pos == NT * PB

    sT = np.ascontiguousarray(
        s_acc.astype(np.float16)
        .reshape(NT, PB, PB).transpose(1, 0, 2).reshape(PB, NT * PB)
    )
    idxLW = _wrap16(np.concatenate(lowE) if lowE else np.empty(0, np.int64))
    idxHW = _wrap16(np.concatenate(highE) if highE else np.empty(0, np.int64))
    # bias laid out [PB, NB*OUT_C] so one DMA preloads all slots
    bias_w = np.ascontiguousarray(
        bias_c.reshape(NB, PB, OUT_C).transpose(1, 0, 2).reshape(PB, NB * OUT_C)
    )
    return sT, idxLW, idxHW, bias_w


# ---------------------------------------------------------------- device build
def _build(NB, TPBL, TPBH, n_in, nbatch, in_c, out_c, gch, reps=1,
           timing=False):
    import concourse.bacc as bacc
    import concourse.mybir as mybir
    from concourse.tile import TileContext

    f32 = mybir.dt.float32
    f16 = mybir.dt.float16
    i16 = mybir.dt.int16
    C2 = nbatch * in_c  # gathered row width (both batches)
    NL, NH = sum(TPBL), sum(TPBH)
    NT = NL + NH
    NK = in_c // PB  # K-chunks in the weight projection

    nc = bacc.Bacc("TRN2", target_bir_lowering=False, debug=False,
                   num_devices=N_CORES, num_swdge_queues=NSWQ)

    # timing builds keep the big dense tensors device-resident (Internal,
    # garbage contents): identical DMA/gather pattern and addresses, but no
    # upload, so wall-clock reps-deltas aren't swamped by axon noise.
    big_kind = "Internal" if timing else "ExternalInput"
    xi_d = nc.dram_tensor("xi", [n_in, C2], f16, kind=big_kind)
    s_d = nc.dram_tensor("sT", [PB, NT * PB], f16, kind=big_kind)
    idxl_d = nc.dram_tensor("idxLW", [PB, max(NL * 8, 8)], i16, kind="ExternalInput")
    idxh_d = nc.dram_tensor("idxHW", [PB, max(NH * 8, 8)], i16, kind="ExternalInput")
    bias_d = nc.dram_tensor("biasC", [PB, NB * out_c], f32, kind="ExternalInput")
    w_d = nc.dram_tensor("wT", [PB, NK * out_c], f32, kind="ExternalInput")
    out_d = nc.dram_tensor("out", [nbatch, NB * PB, out_c], f32,
                           kind="ExternalOutput")

    # processing order: per slot, its low tiles then its high tiles.
    # each entry: (slot, first, last, stream ('l'/'h'), stream tile index)
    sched = []
    nl = nh = 0
    for s in range(NB):
        tpb = TPBL[s] + TPBH[s]
        t = 0
        for _ in range(TPBL[s]):
            sched.append((s, t == 0, t == tpb - 1, "l", nl))
            nl += 1
            t += 1
        for _ in range(TPBH[s]):
            sched.append((s, t == 0, t == tpb - 1, "h", nh))
            nh += 1
            t += 1

    def body(nc, tc, pools):
        (cpool, gl_pool, gh_pool, spool, segpool, opool,
         papool, popool) = pools
        w_sb = cpool.tile([PB, NK * out_c], f32, tag="w")
        idxl_sb = cpool.tile([PB, max(NL * 8, 8)], i16, tag="idxl")
        idxh_sb = cpool.tile([PB, max(NH * 8, 8)], i16, tag="idxh")
        bias_sb = cpool.tile([PB, NB * out_c], f32, tag="bias")
        nc.sync.dma_start(out=w_sb[:], in_=w_d[:])
        nc.sync.dma_start(out=idxl_sb[:], in_=idxl_d[:])
        nc.sync.dma_start(out=idxh_sb[:], in_=idxh_d[:])
        nc.sync.dma_start(out=bias_sb[:], in_=bias_d[:])

        qctr = [0]

        def gather(pool, tag, idx_sb, src_ap, t0, nstream):
            n = min(gch, nstream - t0)
            g = pool.tile([PB, gch * C2], f16, tag=tag)
            nc.gpsimd.dma_gather(
                out_ap=g[:, : n * C2].rearrange("p (t e) -> p t e", e=C2),
                in_ap=src_ap,
                idxs_ap=idx_sb[:, t0 * 8 : (t0 + n) * 8],
                num_idxs=n * PB,
                num_idxs_reg=n * PB,
                elem_size=C2,
                queue_num=qctr[0] % NSWQ,
            )
            qctr[0] += 1
            return g

        SCH = 16  # S-stream chunk (tiles per dma_start)
        gl = gh = None
        s_sb = None
        pacc = None
        cur_s = -1
        for j in range(NT):
            s, first, last, stream, st = sched[j]
            if j % SCH == 0:
                n = min(SCH, NT - j)
                s_sb = spool.tile([PB, SCH * PB], f16, tag="s")
                nc.sync.dma_start(
                    out=s_sb[:, : n * PB],
                    in_=s_d[:, j * PB : (j + n) * PB],
                )
            if stream == "l":
                if st % gch == 0:
                    gl = gather(gl_pool, "gl", idxl_sb, xi_d[:], st, NL)
                g, off = gl, st % gch
            else:
                if st % gch == 0:
                    gh = gather(gh_pool, "gh", idxh_sb, xi_d[H16:, :], st, NH)
                g, off = gh, st % gch
            if first:
                cur_s = s
                pacc = [papool.tile([PB, PB], f32, name=f"pacc{k}",
                                    tag=f"pacc{k}", bufs=2 if k < 2 else 1)
                        for k in range(C2 // PB)]

            # transposed orientation: pacc[ch, r] += G^T @ S^T per 128-ch
            # chunk, so the segment sum lands CHANNEL-MAJOR and the weight
            # projection can consume SBUF slices as lhsT directly (no PE
            # transposes, no per-chunk PSUM->SBUF copies).
            soff = j % SCH
            for k in range(C2 // PB):
                nc.tensor.matmul(
                    out=pacc[k][:],
                    lhsT=g[:, off * C2 + k * PB : off * C2 + (k + 1) * PB],
                    rhs=s_sb[:, soff * PB : (soff + 1) * PB],
                    start=first,
                    stop=last,
                )

            if last:
                seg = segpool.tile([PB, C2], f32, tag="seg")
                for k in range(C2 // PB):
                    if k % 2 == 0:
                        nc.scalar.copy(
                            out=seg[:, k * PB : (k + 1) * PB], in_=pacc[k][:]
                        )
                    else:
                        nc.vector.tensor_copy(
                            out=seg[:, k * PB : (k + 1) * PB], in_=pacc[k][:]
                        )
                for b in range(nbatch):
                    po = popool.tile([PB, out_c], f32)
                    for k in range(NK):
                        nc.tensor.matmul(
                            out=po[:],
                            lhsT=seg[:, (b * NK + k) * PB : (b * NK + k + 1) * PB],
                            rhs=w_sb[:, k * out_c : (k + 1) * out_c],
                            start=(k == 0),
                            stop=(k == NK - 1),
                        )
                    osb = opool.tile([PB, out_c], f32, tag="o")
                    nc.vector.tensor_tensor(
                        out=osb[:], in0=po[:],
                        in1=bias_sb[:, cur_s * out_c : (cur_s + 1) * out_c],
                        op=mybir.AluOpType.add,
                    )
                    nc.sync.dma_start(
                        out=out_d[b, cur_s * PB : (cur_s + 1) * PB, :],
                        in_=osb[:],
                    )

    with TileContext(nc) as tc:
        with (
            tc.tile_pool(name="const", bufs=1) as cpool,
            tc.tile_pool(name="gl", bufs=6) as gl_pool,
            tc.tile_pool(name="gh", bufs=6) as gh_pool,
            tc.tile_pool(name="s", bufs=4) as spool,
            tc.tile_pool(name="seg", bufs=2) as segpool,
            tc.tile_pool(name="o", bufs=4) as opool,
            tc.tile_pool(name="pacc", bufs=1, space="PSUM") as papool,
            tc.tile_pool(name="pout", bufs=2, space="PSUM") as popool,
        ):
            pools = (cpool, gl_pool, gh_pool, spool, segpool, opool,
                     papool, popool)
            if reps == 1:
                body(nc, tc, pools)
            else:
                with tc.For_i(0, reps, 1):
                    body(nc, tc, pools)

    nc.compile()
    return nc


def _host_arrays(x, weight):
    xi = np.ascontiguousarray(
        np.concatenate([x[b] for b in range(B)], axis=1).astype(np.float16)
    )
    NK = IN_C // PB
    wT = np.ascontiguousarray(
        np.concatenate([weight[k * PB : (k + 1) * PB] for k in range(NK)], axis=1)
    )
    return xi, wT


def _in_maps(rows, cols, vals, weight, bias, x, plan):
    NB, TPBL, TPBH, rowsets = plan
    xi, wT = _host_arrays(x, weight)
    order_r = np.argsort(rows, kind="stable")
    bnd_r = np.searchsorted(rows[order_r], np.arange(N_OUT + 1))
    maps = []
    for c in range(N_CORES):
        sT, idxLW, idxHW, bias_c = _pack_core(
            c, plan, rows, cols, vals, bias, order_r, bnd_r
        )
        if idxLW.size == 0:
            idxLW = np.zeros((PB, 8), np.int16)
        if idxHW.size == 0:
            idxHW = np.zeros((PB, 8), np.int16)
        maps.append(
            {
                "xi": xi,
                "sT": sT,
                "idxLW": idxLW,
                "idxHW": idxHW,
                "biasC": bias_c,
                "wT": wT,
            }
        )
    return maps


def time_hw(inputs, reps=(1, 2049), trials=8):
    """HW ns/iter via wall-clock delta between For_i repeat-count variants.
    Timing builds keep xi/sT Internal (no upload) so dispatch noise is
    small; remaining per-call costs are identical across variants and
    cancel."""
    import time as _time

    from concourse.bass_utils import run_bass_kernel_spmd

    rows = np.asarray(inputs["rows"], dtype=np.int64)
    cols = np.asarray(inputs["cols"], dtype=np.int64)
    vals = np.asarray(inputs["vals"], dtype=np.float32)
    x = np.asarray(inputs["x"], dtype=np.float32)
    weight = np.asarray(inputs["weight"], dtype=np.float32)
    bias = np.asarray(inputs["bias"], dtype=np.float32)

    plan = _plan(rows, cols)
    NB, TPBL, TPBH, rowsets = plan
    maps = _in_maps(rows, cols, vals, weight, bias, x, plan)
    tmaps = [{k: v for k, v in m.items() if k not in ("xi", "sT")}
             for m in maps]

    r1, r2 = min(reps), max(reps)
    ncs = {}
    for r in reps:
        ncs[r] = _build(NB, TPBL, TPBH, N_IN, B, IN_C, OUT_C, GCH, reps=r,
                        timing=True)
        run_bass_kernel_spmd(ncs[r], tmaps, core_ids=list(range(N_CORES)))

    # axon wall-clock noise is strongly time-correlated, so interleave the
    # two variants and difference ADJACENT calls; median of paired deltas.
    deltas = []
    pairs = []
    for _ in range(trials):
        t0 = _time.perf_counter()
        run_bass_kernel_spmd(ncs[r1], tmaps, core_ids=list(range(N_CORES)))
        t1 = _time.perf_counter()
        run_bass_kernel_spmd(ncs[r2], tmaps, core_ids=list(range(N_CORES)))
        t2 = _time.perf_counter()
        pairs.append((t1 - t0, t2 - t1))
        deltas.append(((t2 - t1) - (t1 - t0)) / (r2 - r1) * 1e9)
    print("pairs:", [(f"{a*1e3:.0f}", f"{b*1e3:.0f}") for a, b in pairs],
          flush=True)
    print("deltas(ns/iter):", [f"{d:.0f}" for d in deltas], flush=True)
    return float(np.median(deltas))


# ---------------------------------------------------------------- entry point
def kernel(x, rows, cols, vals, weight, bias):
    global LAST_RESULTS
    from concourse.bass_utils import run_bass_kernel_spmd

    x = np.asarray(x, dtype=np.float32)
    rows = np.asarray(rows, dtype=np.int64)
    cols = np.asarray(cols, dtype=np.int64)
    vals = np.asarray(vals, dtype=np.float32)
    weight = np.asarray(weight, dtype=np.float32)
    bias = np.asarray(bias, dtype=np.float32)

    plan = _plan(rows, cols)
    NB, TPBL, TPBH, rowsets = plan

    key = (NB, tuple(TPBL), tuple(TPBH), GCH)
    if key not in _CACHE:
        _CACHE.clear()
        _CACHE[key] = _build(NB, TPBL, TPBH, N_IN, B, IN_C, OUT_C, GCH)
    nc = _CACHE[key]

    maps = _in_maps(rows, cols, vals, weight, bias, x, plan)
    res = run_bass_kernel_spmd(nc, maps, core_ids=list(range(N_CORES)))
    LAST_RESULTS = res

    out = np.empty((B, N_OUT, OUT_C), dtype=np.float32)
    for c in range(N_CORES):
        oc = res.results[c]["out"]
        for s in range(NB):
            rowlist = rowsets[c][s]
            if rowlist is None or len(rowlist) == 0:
                continue
            out[:, rowlist, :] = oc[:, s * PB : s * PB + len(rowlist), :]
    return out


# revision 32
# speedup vs baseline: 1.0032x; 1.0032x over previous
"""Trainium2 Bass kernel for batched GNN message passing.

Computes, for each batch b:
    ax[b] = segment_sum(x[b][cols] * vals[:, None], rows, num_segments=N_OUT)
    out[b] = ax[b] @ weight + bias

Strategy (8 NeuronCores):
  * The two batches share one graph, so x is interleaved host-side into
    xi[n] = concat(x[0,n], x[1,n]) -> [N_IN, 2*IN_C] fp16; one gathered
    row serves both batches.
  * Output rows are split into 128-row blocks, dealt to the 8 cores so
    every core runs an identical program: NB block-slots, slot s
    processing TPBL[s] + TPBH[s] tiles of 128 edges (dma_gather indices
    are int16, so edges are split into col < 32768 gathered from xi[0:]
    and col >= 32768 gathered from xi[32768:]).
  * Per 128-edge tile: dma_gather of 128 rows (1KB fp16 each) from HBM,
    then per 128-channel chunk a PE matmul pacc[ch, r] += G^T @ S^T
    accumulated over the slot's tiles (the segment sum, kept CHANNEL-
    MAJOR).  The scaled selection matrices S^T[e, r] = vals[e] *
    (rloc[e] == r) are PRECOMPUTED ON HOST in fp16 and streamed from HBM
    (dense DMA), because per-tile DVE builds cost ~5us each on HW and
    serialized the whole kernel.
  * Per slot epilogue: one PSUM->SBUF copy; the channel-major layout
    feeds the weight projection directly as lhsT (no transposes), then
    bias add (DVE) and DMA out.
"""

import os as _os

import numpy as np

# ---------------------------------------------------------------- problem dims
B = 2
N_IN = 50000
N_OUT = 12500
NNZ = 500000
IN_C = 256
OUT_C = 256
N_CORES = 8
PB = 128  # rows per output block == partition count
H16 = 32768  # int16 index limit for dma_gather

GCH = int(_os.environ.get("K_GCH", "8"))  # 128-edge tiles per dma_gather call
# SWDGE queues: gather descriptor-gen spread over NSWQ Q7 core pairs
NSWQ = int(_os.environ.get("K_NSWQ", "4"))

_CACHE = {}
LAST_RESULTS = None


# ---------------------------------------------------------------- host planning
def _plan(rows, cols):
    """Pack output rows into (core, slot) bins of <=128 rows, balancing the
    low/high edge loads so every slot's tile counts are tight and uniform.
    Slot s everywhere holds TPBL[s] low tiles + TPBH[s] high tiles."""
    Lr = np.bincount(rows[cols < H16], minlength=N_OUT)
    Hr = np.bincount(rows[cols >= H16], minlength=N_OUT)
    NB = -(-N_OUT // (PB * N_CORES))
    nbins = N_CORES * NB
    tL = max(Lr.sum() / nbins, 1.0)
    tH = max(Hr.sum() / nbins, 1.0)

    order = np.argsort(-(Lr + Hr), kind="stable")
    binL = np.zeros(nbins)
    binH = np.zeros(nbins)
    binN = np.zeros(nbins, dtype=np.int64)
    bin_rows = [[] for _ in range(nbins)]
    for r in order:
        score = np.maximum((binL + Lr[r]) / tL, (binH + Hr[r]) / tH)
        score[binN >= PB] = np.inf
        b = int(score.argmin())
        binL[b] += Lr[r]
        binH[b] += Hr[r]
        binN[b] += 1
        bin_rows[b].append(int(r))

    # group bins into slots by (L-quantum, H) so per-slot maxima stay tight
    q = np.lexsort((-binH, -(-(-binL.astype(np.int64) // PB))))
    rowsets = [[None] * NB for _ in range(N_CORES)]
    for s in range(NB):
        grp = q[s * N_CORES : (s + 1) * N_CORES]
        for c, b in enumerate(grp):
            rowsets[c][s] = np.array(sorted(bin_rows[b]), dtype=np.int64)

    # tile counts from DISTINCT cols per (core, slot): repeated cols within
    # a slot are folded into one gathered row (S accumulates their vals)
    order_r = np.argsort(rows, kind="stable")
    bnd_r = np.searchsorted(rows[order_r], np.arange(N_OUT + 1))
    TPBL, TPBH = [], []
    for s in range(NB):
        tl = th = 0
        for c in range(N_CORES):
            lo, _, hi, _ = _slot_edges(rowsets[c][s], rows, cols, order_r,
                                       bnd_r)
            nlo = len(np.unique(cols[lo])) if len(lo) else 0
            nhi = len(np.unique(cols[hi])) if len(hi) else 0
            tl = max(tl, -(-nlo // PB))
            th = max(th, -(-nhi // PB))
        if tl + th == 0:
            tl = 1
        TPBL.append(tl)
        TPBH.append(th)
    return NB, TPBL, TPBH, rowsets


def _slot_edges(rowlist, rows, cols, order_r, bnd_r):
    """Edge ids + local row indices for one (core, slot), split lo/hi."""
    if rowlist is None or len(rowlist) == 0:
        e = np.empty(0, np.int64)
        return e, e, e, e
    per_row = [order_r[bnd_r[r] : bnd_r[r + 1]] for r in rowlist]
    eids = (np.concatenate(per_row) if per_row else np.empty(0, np.int64))
    rloc = np.repeat(np.arange(len(rowlist)), [len(e) for e in per_row])
    m = cols[eids] < H16
    return eids[m], rloc[m], eids[~m], rloc[~m]


def _wrap16(flat):
    """int16 index stream -> dma_gather layout: idx i at partition i%16,
    col i//16, replicated across the 8 gpsimd core groups."""
    n = len(flat)
    assert n % 16 == 0
    w = np.ascontiguousarray(flat.reshape(n // 16, 16).T.astype(np.int16))
    return np.ascontiguousarray(np.tile(w, (8, 1)))


def _pack_core(c, plan, rows, cols, vals, bias, order_r, bnd_r):
    """Per-core arrays: sT [128, NT*128] fp16 selection matrices (tile j at
    columns j*128:(j+1)*128, partition = edge-in-tile), wrapped int16
    gather index streams, bias."""
    NB, TPBL, TPBH, rowsets = plan
    NT = sum(TPBL) + sum(TPBH)
    s_acc = np.zeros(NT * PB * PB, dtype=np.float32)
    lowE, highE = [], []
    bias_c = np.zeros((NB * PB, OUT_C), dtype=np.float32)
    pos = 0
    for s in range(NB):
        rowlist = rowsets[c][s]
        nr = len(rowlist)
        lo, lo_rl, hi, hi_rl = _slot_edges(rowlist, rows, cols, order_r,
                                           bnd_r)
        if nr:
            bias_c[s * PB : s * PB + nr] = bias[rowlist]
        for lst, rl, tpb, base, acc in (
            (lo, lo_rl, TPBL[s], 0, lowE),
            (hi, hi_rl, TPBH[s], H16, highE),
        ):
            k = tpb * PB
            if k == 0:
                assert len(lst) == 0
                continue
            # gather DISTINCT cols once; S row accumulates all edge vals
            dcols, inv = np.unique(cols[lst], return_inverse=True)
            ne = len(dcols)
            assert ne <= k, (ne, k)
            np.add.at(s_acc, (pos + inv) * PB + rl, vals[lst])
            cc = np.zeros(k, dtype=np.int64)
            cc[:ne] = dcols - base
            acc.append(cc)
            pos += k
    assert pos == NT * PB

    sT = np.ascontiguousarray(
        s_acc.astype(np.float16)
        .reshape(NT, PB, PB).transpose(1, 0, 2).reshape(PB, NT * PB)
    )
    idxLW = _wrap16(np.concatenate(lowE) if lowE else np.empty(0, np.int64))
    idxHW = _wrap16(np.concatenate(highE) if highE else np.empty(0, np.int64))
    # bias laid out [PB, NB*OUT_C] so one DMA preloads all slots
    bias_w = np.ascontiguousarray(
        bias_c.reshape(NB, PB, OUT_C).transpose(1, 0, 2).reshape(PB, NB * OUT_C)
    )
    return sT, idxLW, idxHW, bias_w


# ---------------------------------------------------------------- device build
def _build(NB, TPBL, TPBH, n_in, nbatch, in_c, out_c, gch, reps=1,
           timing=False):
    import concourse.bacc as bacc
    import concourse.mybir as mybir
    from concourse.tile import TileContext

    f32 = mybir.dt.float32
    f16 = mybir.dt.float16
    i16 = mybir.dt.int16
    C2 = nbatch * in_c  # gathered row width (both batches)
    NL, NH = sum(TPBL), sum(TPBH)
    NT = NL + NH
    NK = in_c // PB  # K-chunks in the weight projection

    nc = bacc.Bacc("TRN2", target_bir_lowering=False, debug=False,
                   num_devices=N_CORES, num_swdge_queues=NSWQ)

    # timing builds keep the big dense tensors device-resident (Internal,
    # garbage contents): identical DMA/gather pattern and addresses, but no
    # upload, so wall-clock reps-deltas aren't swamped by axon noise.
    big_kind = "Internal" if timing else "ExternalInput"
    xi_d = nc.dram_tensor("xi", [n_in, C2], f16, kind=big_kind)
    s_d = nc.dram_tensor("sT", [PB, NT * PB], f16, kind=big_kind)
    idxl_d = nc.dram_tensor("idxLW", [PB, max(NL * 8, 8)], i16, kind="ExternalInput")
    idxh_d = nc.dram_tensor("idxHW", [PB, max(NH * 8, 8)], i16, kind="ExternalInput")
    bias_d = nc.dram_tensor("biasC", [PB, NB * out_c], f32, kind="ExternalInput")
    w_d = nc.dram_tensor("wT", [PB, NK * out_c], f32, kind="ExternalInput")
    out_d = nc.dram_tensor("out", [nbatch, NB * PB, out_c], f32,
                           kind="ExternalOutput")

    # processing order: per slot, its low tiles then its high tiles.
    # each entry: (slot, first, last, stream ('l'/'h'), stream tile index)
    sched = []
    nl = nh = 0
    for s in range(NB):
        tpb = TPBL[s] + TPBH[s]
        t = 0
        for _ in range(TPBL[s]):
            sched.append((s, t == 0, t == tpb - 1, "l", nl))
            nl += 1
            t += 1
        for _ in range(TPBH[s]):
            sched.append((s, t == 0, t == tpb - 1, "h", nh))
            nh += 1
            t += 1

    def body(nc, tc, pools):
        (cpool, gl_pool, gh_pool, spool, segpool, opool,
         papool, popool) = pools
        w_sb = cpool.tile([PB, NK * out_c], f32, tag="w")
        idxl_sb = cpool.tile([PB, max(NL * 8, 8)], i16, tag="idxl")
        idxh_sb = cpool.tile([PB, max(NH * 8, 8)], i16, tag="idxh")
        bias_sb = cpool.tile([PB, NB * out_c], f32, tag="bias")
        nc.sync.dma_start(out=w_sb[:], in_=w_d[:])
        nc.sync.dma_start(out=idxl_sb[:], in_=idxl_d[:])
        nc.sync.dma_start(out=idxh_sb[:], in_=idxh_d[:])
        nc.sync.dma_start(out=bias_sb[:], in_=bias_d[:])

        qctr = [0]

        def gather(pool, tag, idx_sb, src_ap, t0, nstream):
            n = min(gch, nstream - t0)
            g = pool.tile([PB, gch * C2], f16, tag=tag)
            nc.gpsimd.dma_gather(
                out_ap=g[:, : n * C2].rearrange("p (t e) -> p t e", e=C2),
                in_ap=src_ap,
                idxs_ap=idx_sb[:, t0 * 8 : (t0 + n) * 8],
                num_idxs=n * PB,
                num_idxs_reg=n * PB,
                elem_size=C2,
                queue_num=qctr[0] % NSWQ,
            )
            qctr[0] += 1
            return g

        SCH = 16  # S-stream chunk (tiles per dma_start)
        gl = gh = None
        s_sb = None
        pacc = None
        cur_s = -1
        for j in range(NT):
            s, first, last, stream, st = sched[j]
            if j % SCH == 0:
                n = min(SCH, NT - j)
                s_sb = spool.tile([PB, SCH * PB], f16, tag="s")
                nc.sync.dma_start(
                    out=s_sb[:, : n * PB],
                    in_=s_d[:, j * PB : (j + n) * PB],
                )
            if stream == "l":
                if st % gch == 0:
                    gl = gather(gl_pool, "gl", idxl_sb, xi_d[:], st, NL)
                g, off = gl, st % gch
            else:
                if st % gch == 0:
                    gh = gather(gh_pool, "gh", idxh_sb, xi_d[H16:, :], st, NH)
                g, off = gh, st % gch
            if first:
                cur_s = s
                pacc = [papool.tile([PB, PB], f32, name=f"pacc{k}",
                                    tag=f"pacc{k}", bufs=2 if k < 2 else 1)
                        for k in range(C2 // PB)]

            # transposed orientation: pacc[ch, r] += G^T @ S^T per 128-ch
            # chunk, so the segment sum lands CHANNEL-MAJOR and the weight
            # projection can consume SBUF slices as lhsT directly (no PE
            # transposes, no per-chunk PSUM->SBUF copies).
            soff = j % SCH
            for k in range(C2 // PB):
                nc.tensor.matmul(
                    out=pacc[k][:],
                    lhsT=g[:, off * C2 + k * PB : off * C2 + (k + 1) * PB],
                    rhs=s_sb[:, soff * PB : (soff + 1) * PB],
                    start=first,
                    stop=last,
                )

            if last:
                seg = segpool.tile([PB, C2], f32, tag="seg")
                for k in range(C2 // PB):
                    if k % 2 == 0:
                        nc.scalar.copy(
                            out=seg[:, k * PB : (k + 1) * PB], in_=pacc[k][:]
                        )
                    else:
                        nc.vector.tensor_copy(
                            out=seg[:, k * PB : (k + 1) * PB], in_=pacc[k][:]
                        )
                for b in range(nbatch):
                    po = popool.tile([PB, out_c], f32)
                    for k in range(NK):
                        nc.tensor.matmul(
                            out=po[:],
                            lhsT=seg[:, (b * NK + k) * PB : (b * NK + k + 1) * PB],
                            rhs=w_sb[:, k * out_c : (k + 1) * out_c],
                            start=(k == 0),
                            stop=(k == NK - 1),
                        )
                    osb = opool.tile([PB, out_c], f32, tag="o")
                    nc.vector.tensor_tensor(
                        out=osb[:], in0=po[:],
                        in1=bias_sb[:, cur_s * out_c : (cur_s + 1) * out_c],
                        op=mybir.AluOpType.add,
                    )
                    nc.sync.dma_start(
                        out=out_d[b, cur_s * PB : (cur_s + 1) * PB, :],
                        in_=osb[:],
                    )

    with TileContext(nc) as tc:
        with (
            tc.tile_pool(name="const", bufs=1) as cpool,
            tc.tile_pool(name="gl", bufs=4) as gl_pool,
            tc.tile_pool(name="gh", bufs=4) as gh_pool,
            tc.tile_pool(name="s", bufs=4) as spool,
            tc.tile_pool(name="seg", bufs=2) as segpool,
            tc.tile_pool(name="o", bufs=4) as opool,
            tc.tile_pool(name="pacc", bufs=1, space="PSUM") as papool,
            tc.tile_pool(name="pout", bufs=2, space="PSUM") as popool,
        ):
            pools = (cpool, gl_pool, gh_pool, spool, segpool, opool,
                     papool, popool)
            if reps == 1:
                body(nc, tc, pools)
            else:
                with tc.For_i(0, reps, 1):
                    body(nc, tc, pools)

    nc.compile()
    return nc


def _host_arrays(x, weight):
    xi = np.ascontiguousarray(
        np.concatenate([x[b] for b in range(B)], axis=1).astype(np.float16)
    )
    NK = IN_C // PB
    wT = np.ascontiguousarray(
        np.concatenate([weight[k * PB : (k + 1) * PB] for k in range(NK)], axis=1)
    )
    return xi, wT


def _in_maps(rows, cols, vals, weight, bias, x, plan):
    NB, TPBL, TPBH, rowsets = plan
    xi, wT = _host_arrays(x, weight)
    order_r = np.argsort(rows, kind="stable")
    bnd_r = np.searchsorted(rows[order_r], np.arange(N_OUT + 1))
    maps = []
    for c in range(N_CORES):
        sT, idxLW, idxHW, bias_c = _pack_core(
            c, plan, rows, cols, vals, bias, order_r, bnd_r
        )
        if idxLW.size == 0:
            idxLW = np.zeros((PB, 8), np.int16)
        if idxHW.size == 0:
            idxHW = np.zeros((PB, 8), np.int16)
        maps.append(
            {
                "xi": xi,
                "sT": sT,
                "idxLW": idxLW,
                "idxHW": idxHW,
                "biasC": bias_c,
                "wT": wT,
            }
        )
    return maps


def time_hw(inputs, reps=(1, 2049), trials=8):
    """HW ns/iter via wall-clock delta between For_i repeat-count variants.
    Timing builds keep xi/sT Internal (no upload) so dispatch noise is
    small; remaining per-call costs are identical across variants and
    cancel."""
    import time as _time

    from concourse.bass_utils import run_bass_kernel_spmd

    rows = np.asarray(inputs["rows"], dtype=np.int64)
    cols = np.asarray(inputs["cols"], dtype=np.int64)
    vals = np.asarray(inputs["vals"], dtype=np.float32)
    x = np.asarray(inputs["x"], dtype=np.float32)
    weight = np.asarray(inputs["weight"], dtype=np.float32)
    bias = np.asarray(inputs["bias"], dtype=np.float32)

    plan = _plan(rows, cols)
    NB, TPBL, TPBH, rowsets = plan
    maps = _in_maps(rows, cols, vals, weight, bias, x, plan)
    tmaps = [{k: v for k, v in m.items() if k not in ("xi", "sT")}
             for m in maps]

    r1, r2 = min(reps), max(reps)
    ncs = {}
    for r in reps:
        ncs[r] = _build(NB, TPBL, TPBH, N_IN, B, IN_C, OUT_C, GCH, reps=r,
                        timing=True)
        run_bass_kernel_spmd(ncs[r], tmaps, core_ids=list(range(N_CORES)))

    # axon wall-clock noise is strongly time-correlated, so interleave the
    # two variants and difference ADJACENT calls; median of paired deltas.
    deltas = []
    pairs = []
    for _ in range(trials):
        t0 = _time.perf_counter()
        run_bass_kernel_spmd(ncs[r1], tmaps, core_ids=list(range(N_CORES)))
        t1 = _time.perf_counter()
        run_bass_kernel_spmd(ncs[r2], tmaps, core_ids=list(range(N_CORES)))
        t2 = _time.perf_counter()
        pairs.append((t1 - t0, t2 - t1))
        deltas.append(((t2 - t1) - (t1 - t0)) / (r2 - r1) * 1e9)
    print("pairs:", [(f"{a*1e3:.0f}", f"{b*1e3:.0f}") for a, b in pairs],
          flush=True)
    print("deltas(ns/iter):", [f"{d:.0f}" for d in deltas], flush=True)
    return float(np.median(deltas))


# ---------------------------------------------------------------- entry point
def kernel(x, rows, cols, vals, weight, bias):
    global LAST_RESULTS
    from concourse.bass_utils import run_bass_kernel_spmd

    x = np.asarray(x, dtype=np.float32)
    rows = np.asarray(rows, dtype=np.int64)
    cols = np.asarray(cols, dtype=np.int64)
    vals = np.asarray(vals, dtype=np.float32)
    weight = np.asarray(weight, dtype=np.float32)
    bias = np.asarray(bias, dtype=np.float32)

    plan = _plan(rows, cols)
    NB, TPBL, TPBH, rowsets = plan

    key = (NB, tuple(TPBL), tuple(TPBH), GCH)
    if key not in _CACHE:
        _CACHE.clear()
        _CACHE[key] = _build(NB, TPBL, TPBH, N_IN, B, IN_C, OUT_C, GCH)
    nc = _CACHE[key]

    maps = _in_maps(rows, cols, vals, weight, bias, x, plan)
    res = run_bass_kernel_spmd(nc, maps, core_ids=list(range(N_CORES)))
    LAST_RESULTS = res

    out = np.empty((B, N_OUT, OUT_C), dtype=np.float32)
    for c in range(N_CORES):
        oc = res.results[c]["out"]
        for s in range(NB):
            rowlist = rowsets[c][s]
            if rowlist is None or len(rowlist) == 0:
                continue
            out[:, rowlist, :] = oc[:, s * PB : s * PB + len(rowlist), :]
    return out


# revision 33
# speedup vs baseline: 1.0335x; 1.0302x over previous
"""Trainium2 Bass kernel for batched GNN message passing.

Computes, for each batch b:
    ax[b] = segment_sum(x[b][cols] * vals[:, None], rows, num_segments=N_OUT)
    out[b] = ax[b] @ weight + bias

Strategy (8 NeuronCores):
  * The two batches share one graph, so x is interleaved host-side into
    xi[n] = concat(x[0,n], x[1,n]) -> [N_IN, 2*IN_C] fp16; one gathered
    row serves both batches.
  * Output rows are split into 128-row blocks, dealt to the 8 cores so
    every core runs an identical program: NB block-slots, slot s
    processing TPBL[s] + TPBH[s] tiles of 128 edges (dma_gather indices
    are int16, so edges are split into col < 32768 gathered from xi[0:]
    and col >= 32768 gathered from xi[32768:]).
  * Per 128-edge tile: dma_gather of 128 rows (1KB fp16 each) from HBM,
    then per 128-channel chunk a PE matmul pacc[ch, r] += G^T @ S^T
    accumulated over the slot's tiles (the segment sum, kept CHANNEL-
    MAJOR).  The scaled selection matrices S^T[e, r] = vals[e] *
    (rloc[e] == r) are PRECOMPUTED ON HOST in fp16 and streamed from HBM
    (dense DMA), because per-tile DVE builds cost ~5us each on HW and
    serialized the whole kernel.
  * Per slot epilogue: one PSUM->SBUF copy; the channel-major layout
    feeds the weight projection directly as lhsT (no transposes), then
    bias add (DVE) and DMA out.
"""

import os as _os

import numpy as np

# ---------------------------------------------------------------- problem dims
B = 2
N_IN = 50000
N_OUT = 12500
NNZ = 500000
IN_C = 256
OUT_C = 256
N_CORES = 8
PB = 128  # rows per output block == partition count
H16 = 32768  # int16 index limit for dma_gather

GCH = int(_os.environ.get("K_GCH", "8"))  # 128-edge tiles per dma_gather call
# SWDGE queues: gather descriptor-gen spread over NSWQ Q7 core pairs
NSWQ = int(_os.environ.get("K_NSWQ", "4"))

_CACHE = {}
LAST_RESULTS = None


# ---------------------------------------------------------------- host planning
def _plan(rows, cols):
    """Pack output rows into (core, slot) bins of <=128 rows, balancing the
    low/high edge loads so every slot's tile counts are tight and uniform.
    Slot s everywhere holds TPBL[s] low tiles + TPBH[s] high tiles."""
    Lr = np.bincount(rows[cols < H16], minlength=N_OUT)
    Hr = np.bincount(rows[cols >= H16], minlength=N_OUT)
    NB = -(-N_OUT // (PB * N_CORES))
    nbins = N_CORES * NB
    tL = max(Lr.sum() / nbins, 1.0)
    tH = max(Hr.sum() / nbins, 1.0)

    order = np.argsort(-(Lr + Hr), kind="stable")
    binL = np.zeros(nbins)
    binH = np.zeros(nbins)
    binN = np.zeros(nbins, dtype=np.int64)
    bin_rows = [[] for _ in range(nbins)]
    for r in order:
        score = np.maximum((binL + Lr[r]) / tL, (binH + Hr[r]) / tH)
        score[binN >= PB] = np.inf
        b = int(score.argmin())
        binL[b] += Lr[r]
        binH[b] += Hr[r]
        binN[b] += 1
        bin_rows[b].append(int(r))

    # group bins into slots by (L-quantum, H) so per-slot maxima stay tight
    q = np.lexsort((-binH, -(-(-binL.astype(np.int64) // PB))))
    rowsets = [[None] * NB for _ in range(N_CORES)]
    for s in range(NB):
        grp = q[s * N_CORES : (s + 1) * N_CORES]
        for c, b in enumerate(grp):
            rowsets[c][s] = np.array(sorted(bin_rows[b]), dtype=np.int64)

    # tile counts from DISTINCT cols per (core, slot): repeated cols within
    # a slot are folded into one gathered row (S accumulates their vals)
    order_r = np.argsort(rows, kind="stable")
    bnd_r = np.searchsorted(rows[order_r], np.arange(N_OUT + 1))
    TPBL, TPBH = [], []
    for s in range(NB):
        tl = th = 0
        for c in range(N_CORES):
            lo, _, hi, _ = _slot_edges(rowsets[c][s], rows, cols, order_r,
                                       bnd_r)
            nlo = len(np.unique(cols[lo])) if len(lo) else 0
            nhi = len(np.unique(cols[hi])) if len(hi) else 0
            tl = max(tl, -(-nlo // PB))
            th = max(th, -(-nhi // PB))
        if tl + th == 0:
            tl = 1
        TPBL.append(tl)
        TPBH.append(th)
    return NB, TPBL, TPBH, rowsets


def _slot_edges(rowlist, rows, cols, order_r, bnd_r):
    """Edge ids + local row indices for one (core, slot), split lo/hi."""
    if rowlist is None or len(rowlist) == 0:
        e = np.empty(0, np.int64)
        return e, e, e, e
    per_row = [order_r[bnd_r[r] : bnd_r[r + 1]] for r in rowlist]
    eids = (np.concatenate(per_row) if per_row else np.empty(0, np.int64))
    rloc = np.repeat(np.arange(len(rowlist)), [len(e) for e in per_row])
    m = cols[eids] < H16
    return eids[m], rloc[m], eids[~m], rloc[~m]


def _wrap16(flat):
    """int16 index stream -> dma_gather layout: idx i at partition i%16,
    col i//16, replicated across the 8 gpsimd core groups."""
    n = len(flat)
    assert n % 16 == 0
    w = np.ascontiguousarray(flat.reshape(n // 16, 16).T.astype(np.int16))
    return np.ascontiguousarray(np.tile(w, (8, 1)))


def _pack_core(c, plan, rows, cols, vals, bias, order_r, bnd_r):
    """Per-core arrays: sT [128, NT*128] fp16 selection matrices (tile j at
    columns j*128:(j+1)*128, partition = edge-in-tile), wrapped int16
    gather index streams, bias."""
    NB, TPBL, TPBH, rowsets = plan
    NT = sum(TPBL) + sum(TPBH)
    s_acc = np.zeros(NT * PB * PB, dtype=np.float32)
    lowE, highE = [], []
    bias_c = np.zeros((NB * PB, OUT_C), dtype=np.float32)
    pos = 0
    for s in range(NB):
        rowlist = rowsets[c][s]
        nr = len(rowlist)
        lo, lo_rl, hi, hi_rl = _slot_edges(rowlist, rows, cols, order_r,
                                           bnd_r)
        if nr:
            bias_c[s * PB : s * PB + nr] = bias[rowlist]
        for lst, rl, tpb, base, acc in (
            (lo, lo_rl, TPBL[s], 0, lowE),
            (hi, hi_rl, TPBH[s], H16, highE),
        ):
            k = tpb * PB
            if k == 0:
                assert len(lst) == 0
                continue
            # gather DISTINCT cols once; S row accumulates all edge vals
            dcols, inv = np.unique(cols[lst], return_inverse=True)
            ne = len(dcols)
            assert ne <= k, (ne, k)
            np.add.at(s_acc, (pos + inv) * PB + rl, vals[lst])
            cc = np.zeros(k, dtype=np.int64)
            cc[:ne] = dcols - base
            acc.append(cc)
            pos += k
    assert pos == NT * PB

    sT = np.ascontiguousarray(
        s_acc.astype(np.float16)
        .reshape(NT, PB, PB).transpose(1, 0, 2).reshape(PB, NT * PB)
    )
    idxLW = _wrap16(np.concatenate(lowE) if lowE else np.empty(0, np.int64))
    idxHW = _wrap16(np.concatenate(highE) if highE else np.empty(0, np.int64))
    # bias laid out [PB, NB*OUT_C] so one DMA preloads all slots
    bias_w = np.ascontiguousarray(
        bias_c.reshape(NB, PB, OUT_C).transpose(1, 0, 2).reshape(PB, NB * OUT_C)
    )
    return sT, idxLW, idxHW, bias_w


# ---------------------------------------------------------------- device build
def _build(NB, TPBL, TPBH, n_in, nbatch, in_c, out_c, gch, reps=1,
           timing=False):
    import concourse.bacc as bacc
    import concourse.mybir as mybir
    from concourse.tile import TileContext

    f32 = mybir.dt.float32
    f16 = mybir.dt.float16
    i16 = mybir.dt.int16
    C2 = nbatch * in_c  # gathered row width (both batches)
    NL, NH = sum(TPBL), sum(TPBH)
    NT = NL + NH
    NK = in_c // PB  # K-chunks in the weight projection

    nc = bacc.Bacc("TRN2", target_bir_lowering=False, debug=False,
                   num_devices=N_CORES, num_swdge_queues=NSWQ)

    # timing builds keep the big dense tensors device-resident (Internal,
    # garbage contents): identical DMA/gather pattern and addresses, but no
    # upload, so wall-clock reps-deltas aren't swamped by axon noise.
    big_kind = "Internal" if timing else "ExternalInput"
    xi_d = nc.dram_tensor("xi", [n_in, C2], f16, kind=big_kind)
    s_d = nc.dram_tensor("sT", [PB, NT * PB], f16, kind=big_kind)
    idxl_d = nc.dram_tensor("idxLW", [PB, max(NL * 8, 8)], i16, kind="ExternalInput")
    idxh_d = nc.dram_tensor("idxHW", [PB, max(NH * 8, 8)], i16, kind="ExternalInput")
    bias_d = nc.dram_tensor("biasC", [PB, NB * out_c], f32, kind="ExternalInput")
    w_d = nc.dram_tensor("wT", [PB, NK * out_c], f32, kind="ExternalInput")
    ident_d = nc.dram_tensor("ident", [PB, PB], f32, kind="ExternalInput")
    out_d = nc.dram_tensor("out", [nbatch, NB * PB, out_c], f32,
                           kind="ExternalOutput")

    # processing order: per slot, its low tiles then its high tiles.
    # each entry: (slot, first, last, stream ('l'/'h'), stream tile index)
    sched = []
    nl = nh = 0
    for s in range(NB):
        tpb = TPBL[s] + TPBH[s]
        t = 0
        for _ in range(TPBL[s]):
            sched.append((s, t == 0, t == tpb - 1, "l", nl))
            nl += 1
            t += 1
        for _ in range(TPBH[s]):
            sched.append((s, t == 0, t == tpb - 1, "h", nh))
            nh += 1
            t += 1

    def body(nc, tc, pools):
        (cpool, gl_pool, gh_pool, spool, segpool, trpool, opool,
         papool, ptpool, popool) = pools
        w_sb = cpool.tile([PB, NK * out_c], f32, tag="w")
        idxl_sb = cpool.tile([PB, max(NL * 8, 8)], i16, tag="idxl")
        idxh_sb = cpool.tile([PB, max(NH * 8, 8)], i16, tag="idxh")
        bias_sb = cpool.tile([PB, NB * out_c], f32, tag="bias")
        ident_sb = cpool.tile([PB, PB], f32, tag="ident")
        nc.sync.dma_start(out=ident_sb[:], in_=ident_d[:])
        nc.sync.dma_start(out=w_sb[:], in_=w_d[:])
        nc.sync.dma_start(out=idxl_sb[:], in_=idxl_d[:])
        nc.sync.dma_start(out=idxh_sb[:], in_=idxh_d[:])
        nc.sync.dma_start(out=bias_sb[:], in_=bias_d[:])

        qctr = [0]

        def gather(pool, tag, idx_sb, src_ap, t0, nstream):
            n = min(gch, nstream - t0)
            g = pool.tile([PB, gch * C2], f16, tag=tag)
            nc.gpsimd.dma_gather(
                out_ap=g[:, : n * C2].rearrange("p (t e) -> p t e", e=C2),
                in_ap=src_ap,
                idxs_ap=idx_sb[:, t0 * 8 : (t0 + n) * 8],
                num_idxs=n * PB,
                num_idxs_reg=n * PB,
                elem_size=C2,
                queue_num=qctr[0] % NSWQ,
            )
            qctr[0] += 1
            return g

        SCH = 16  # S-stream chunk (tiles per dma_start)
        gl = gh = None
        s_sb = None
        pacc = None
        cur_s = -1
        for j in range(NT):
            s, first, last, stream, st = sched[j]
            if j % SCH == 0:
                n = min(SCH, NT - j)
                s_sb = spool.tile([PB, SCH * PB], f16, tag="s")
                nc.sync.dma_start(
                    out=s_sb[:, : n * PB],
                    in_=s_d[:, j * PB : (j + n) * PB],
                )
            if stream == "l":
                if st % gch == 0:
                    gl = gather(gl_pool, "gl", idxl_sb, xi_d[:], st, NL)
                g, off = gl, st % gch
            else:
                if st % gch == 0:
                    gh = gather(gh_pool, "gh", idxh_sb, xi_d[H16:, :], st, NH)
                g, off = gh, st % gch
            if first:
                cur_s = s
                pacc = papool.tile([PB, C2], f32, name="pacc", tag="pacc",
                                   bufs=2)

            # row-major segment sum: one matmul per tile (lhsT = S loads
            # once, rhs = G streams all 512 channels) keeps PE time low.
            soff = j % SCH
            nc.tensor.matmul(
                out=pacc[:],
                lhsT=s_sb[:, soff * PB : (soff + 1) * PB],
                rhs=g[:, off * C2 : (off + 1) * C2],
                start=first,
                stop=last,
            )

            if last:
                seg = segpool.tile([PB, C2], f32, tag="seg")
                nc.scalar.copy(out=seg[:], in_=pacc[:])
                for b in range(nbatch):
                    trs = []
                    for k in range(NK):
                        ptr = ptpool.tile([PB, PB], f32, name="ptr", tag="ptr")
                        nc.tensor.transpose(
                            out=ptr[:],
                            in_=seg[:, b * in_c + k * PB : b * in_c + (k + 1) * PB],
                            identity=ident_sb[:],
                        )
                        trk = trpool.tile([PB, PB], f32, name="trk", tag="trk")
                        if k % 2 == 0:
                            nc.vector.tensor_copy(out=trk[:], in_=ptr[:])
                        else:
                            nc.scalar.copy(out=trk[:], in_=ptr[:])
                        trs.append(trk)
                    po = popool.tile([PB, out_c], f32)
                    for k in range(NK):
                        nc.tensor.matmul(
                            out=po[:],
                            lhsT=trs[k][:],
                            rhs=w_sb[:, k * out_c : (k + 1) * out_c],
                            start=(k == 0),
                            stop=(k == NK - 1),
                        )
                    osb = opool.tile([PB, out_c], f32, tag="o")
                    nc.vector.tensor_tensor(
                        out=osb[:], in0=po[:],
                        in1=bias_sb[:, cur_s * out_c : (cur_s + 1) * out_c],
                        op=mybir.AluOpType.add,
                    )
                    nc.sync.dma_start(
                        out=out_d[b, cur_s * PB : (cur_s + 1) * PB, :],
                        in_=osb[:],
                    )

    with TileContext(nc) as tc:
        with (
            tc.tile_pool(name="const", bufs=1) as cpool,
            tc.tile_pool(name="gl", bufs=4) as gl_pool,
            tc.tile_pool(name="gh", bufs=4) as gh_pool,
            tc.tile_pool(name="s", bufs=4) as spool,
            tc.tile_pool(name="seg", bufs=2) as segpool,
            tc.tile_pool(name="tr", bufs=4) as trpool,
            tc.tile_pool(name="o", bufs=4) as opool,
            tc.tile_pool(name="pacc", bufs=1, space="PSUM") as papool,
            tc.tile_pool(name="ptr", bufs=2, space="PSUM") as ptpool,
            tc.tile_pool(name="pout", bufs=2, space="PSUM") as popool,
        ):
            pools = (cpool, gl_pool, gh_pool, spool, segpool, trpool, opool,
                     papool, ptpool, popool)
            if reps == 1:
                body(nc, tc, pools)
            else:
                with tc.For_i(0, reps, 1):
                    body(nc, tc, pools)

    nc.compile()
    return nc


def _host_arrays(x, weight):
    xi = np.ascontiguousarray(
        np.concatenate([x[b] for b in range(B)], axis=1).astype(np.float16)
    )
    NK = IN_C // PB
    wT = np.ascontiguousarray(
        np.concatenate([weight[k * PB : (k + 1) * PB] for k in range(NK)], axis=1)
    )
    ident = np.eye(PB, dtype=np.float32)
    return xi, wT, ident


def _in_maps(rows, cols, vals, weight, bias, x, plan):
    NB, TPBL, TPBH, rowsets = plan
    xi, wT, ident = _host_arrays(x, weight)
    order_r = np.argsort(rows, kind="stable")
    bnd_r = np.searchsorted(rows[order_r], np.arange(N_OUT + 1))
    maps = []
    for c in range(N_CORES):
        sT, idxLW, idxHW, bias_c = _pack_core(
            c, plan, rows, cols, vals, bias, order_r, bnd_r
        )
        if idxLW.size == 0:
            idxLW = np.zeros((PB, 8), np.int16)
        if idxHW.size == 0:
            idxHW = np.zeros((PB, 8), np.int16)
        maps.append(
            {
                "xi": xi,
                "sT": sT,
                "idxLW": idxLW,
                "idxHW": idxHW,
                "biasC": bias_c,
                "wT": wT,
                "ident": ident,
            }
        )
    return maps


def time_hw(inputs, reps=(1, 2049), trials=8):
    """HW ns/iter via wall-clock delta between For_i repeat-count variants.
    Timing builds keep xi/sT Internal (no upload) so dispatch noise is
    small; remaining per-call costs are identical across variants and
    cancel."""
    import time as _time

    from concourse.bass_utils import run_bass_kernel_spmd

    rows = np.asarray(inputs["rows"], dtype=np.int64)
    cols = np.asarray(inputs["cols"], dtype=np.int64)
    vals = np.asarray(inputs["vals"], dtype=np.float32)
    x = np.asarray(inputs["x"], dtype=np.float32)
    weight = np.asarray(inputs["weight"], dtype=np.float32)
    bias = np.asarray(inputs["bias"], dtype=np.float32)

    plan = _plan(rows, cols)
    NB, TPBL, TPBH, rowsets = plan
    maps = _in_maps(rows, cols, vals, weight, bias, x, plan)
    tmaps = [{k: v for k, v in m.items() if k not in ("xi", "sT")}
             for m in maps]

    r1, r2 = min(reps), max(reps)
    ncs = {}
    for r in reps:
        ncs[r] = _build(NB, TPBL, TPBH, N_IN, B, IN_C, OUT_C, GCH, reps=r,
                        timing=True)
        run_bass_kernel_spmd(ncs[r], tmaps, core_ids=list(range(N_CORES)))

    # axon wall-clock noise is strongly time-correlated, so interleave the
    # two variants and difference ADJACENT calls; median of paired deltas.
    deltas = []
    pairs = []
    for _ in range(trials):
        t0 = _time.perf_counter()
        run_bass_kernel_spmd(ncs[r1], tmaps, core_ids=list(range(N_CORES)))
        t1 = _time.perf_counter()
        run_bass_kernel_spmd(ncs[r2], tmaps, core_ids=list(range(N_CORES)))
        t2 = _time.perf_counter()
        pairs.append((t1 - t0, t2 - t1))
        deltas.append(((t2 - t1) - (t1 - t0)) / (r2 - r1) * 1e9)
    print("pairs:", [(f"{a*1e3:.0f}", f"{b*1e3:.0f}") for a, b in pairs],
          flush=True)
    print("deltas(ns/iter):", [f"{d:.0f}" for d in deltas], flush=True)
    return float(np.median(deltas))


# ---------------------------------------------------------------- entry point
def kernel(x, rows, cols, vals, weight, bias):
    global LAST_RESULTS
    from concourse.bass_utils import run_bass_kernel_spmd

    x = np.asarray(x, dtype=np.float32)
    rows = np.asarray(rows, dtype=np.int64)
    cols = np.asarray(cols, dtype=np.int64)
    vals = np.asarray(vals, dtype=np.float32)
    weight = np.asarray(weight, dtype=np.float32)
    bias = np.asarray(bias, dtype=np.float32)

    plan = _plan(rows, cols)
    NB, TPBL, TPBH, rowsets = plan

    key = (NB, tuple(TPBL), tuple(TPBH), GCH)
    if key not in _CACHE:
        _CACHE.clear()
        _CACHE[key] = _build(NB, TPBL, TPBH, N_IN, B, IN_C, OUT_C, GCH)
    nc = _CACHE[key]

    maps = _in_maps(rows, cols, vals, weight, bias, x, plan)
    res = run_bass_kernel_spmd(nc, maps, core_ids=list(range(N_CORES)))
    LAST_RESULTS = res

    out = np.empty((B, N_OUT, OUT_C), dtype=np.float32)
    for c in range(N_CORES):
        oc = res.results[c]["out"]
        for s in range(NB):
            rowlist = rowsets[c][s]
            if rowlist is None or len(rowlist) == 0:
                continue
            out[:, rowlist, :] = oc[:, s * PB : s * PB + len(rowlist), :]
    return out
